# revision 11
# baseline (speedup 1.0000x reference)
"""ButterflyLinear Trainium2 kernel.

Math: out[b, s, i] = (sum_o x[b, s, o] * W[o, i]) * mask[s, i], with
mask[s, i] = 1 iff 4s <= i < 4s+4 (stride-4 band). The band makes the
output block-diagonal: s-rows [128t, 128t+128) only touch output columns
[512t, 512t+512) -- an 8x compute reduction vs the full matmul.

Sharding (8 cores): core t owns s-block t for all 16 batches
(tensor-parallel split of W columns; no inter-core communication).

Packing: a 32-row s-sub-block spans a 128-wide band window, identical
for every batch. The moving operand packs FOUR batches (N = 128 = 4
batches x 32 s-rows) against the 128-wide W window as the STATIONARY
(out = window-col x batch-row, i.e. transposed blocks) -- one weight
load serves 4 batch-group matmuls and W streams once per batch QUAD.
Each accumulation chain lives in a QUARTER PSUM bank; all 16 chains
(4 batch-quads x 4 sub-blocks) fit in 4 banks at once: single wave.
PSUM has_written is per-element: only the first matmul of each bank
uses start=True (clears the whole bank); later chains in that bank
start with start=False and overwrite their untouched quarter.

Per-core device program (all fp16 inputs, fp32 PSUM accumulate):
  - 8 o-chunks streamed (W chunk 128KB + x chunk 512KB); the first x
    chunk is split per batch-quad so matmuls start ~2us earlier
  - 128 matmuls (K=128, M=128, N=128), 4 per weight load
  - 4 full-bank copies (alternating VectorE/ScalarE) -> DMA raw blocks
Host extracts the 4-wide diagonal band from the transposed raw blocks
into the zero-filled (16, 1024, 4096) result.
"""

import os
import sys
from contextlib import ExitStack

import numpy as np

if "/opt/trn_rl_repo" not in sys.path:
    sys.path.insert(0, "/opt/trn_rl_repo")

import concourse.bass as bass  # noqa: E402
import concourse.tile as tile  # noqa: E402
from concourse import bacc, mybir  # noqa: E402
from concourse.bass_utils import run_bass_kernel_spmd  # noqa: E402

B = 16  # batch
NT = 8  # s-blocks == cores
SB = 128  # s rows per block
NC_ = 8  # o chunks
KC = 128  # o rows per chunk
NI = 512  # output columns per block
QB = int(os.environ.get("BFK_QB", "4"))  # batches packed per matmul
RW = SB // QB  # s-rows per sub-block
NH = QB  # sub-blocks per s-block (RW*NH = 128)
NW = 4 * RW  # W window per sub-block
NG = B // QB  # batch groups
WSTAT = os.environ.get("BFK_WSTAT", "1") == "1"  # W as stationary
SPLIT0 = os.environ.get("BFK_SPLIT0", "1") == "1"  # split first x chunk

# fp16 (11-bit mantissa) halves DMA traffic vs f32 and streams the PE at
# 1 cycle/row; accumulation stays fp32 in PSUM (band rel err ~3e-4).
_DT_CHOICES = {
    "f16": mybir.dt.float16,
    "f32r": mybir.dt.float32r,
    "f32": mybir.dt.float32,
}
MM_DT = _DT_CHOICES[os.environ.get("BFK_DT", "f16")]
F32 = mybir.dt.float32

_STATE: dict = {}


def _build():
    if "nc" in _STATE:
        return _STATE["nc"]

    nc = bacc.Bacc(
        "TRN2", target_bir_lowering=False, debug=False, num_devices=NT
    )
    xt = nc.dram_tensor(
        "xt", [NC_, KC, NG, NH, SB], MM_DT, kind="ExternalInput"
    ).ap()
    wt = nc.dram_tensor("wt", [NC_, KC, NH, NW], MM_DT, kind="ExternalInput").ap()
    out = nc.dram_tensor("out", [NG, SB, NI], F32, kind="ExternalOutput").ap()

    with tile.TileContext(nc) as tc, ExitStack() as ctx:
        wp = ctx.enter_context(tc.tile_pool(name="w", bufs=1))
        xp = ctx.enter_context(tc.tile_pool(name="x", bufs=1))
        pp = ctx.enter_context(tc.tile_pool(name="ps", bufs=8, space="PSUM"))
        op = ctx.enter_context(tc.tile_pool(name="o", bufs=6))

        w_t = []
        x_t = []
        for c in range(NC_):
            w = wp.tile([KC, NH, NW], MM_DT, tag=f"w{c}")
            nc.sync.dma_start(out=w[:], in_=wt[c])
            w_t.append(w)
            xc = xp.tile([KC, NG, NH, SB], MM_DT, tag=f"x{c}")
            if c == 0 and SPLIT0:
                # Per-quad sub-DMAs so the first matmuls start as soon as
                # the first 128KB lands instead of waiting for 512KB.
                for g in range(NG):
                    nc.sync.dma_start(out=xc[:, g], in_=xt[c, :, g])
            else:
                nc.sync.dma_start(out=xc[:], in_=xt[c])
            x_t.append(xc)

        ps = [pp.tile([SB, NI], F32, tag="ps", name=f"ps_{g}") for g in range(NG)]

        if WSTAT:
            # out^T blocks: stationary = W window (o, NW), moving = x pack
            # (o, 128). For c==0, loop g-outer so each batch-quad's chains
            # start as soon as its sub-DMA lands; afterwards h-outer so one
            # weight load serves all 4 quads.
            for g in range(NG):
                for h in range(NH):
                    nc.tensor.matmul(
                        ps[g][:, h * SB : (h + 1) * SB],
                        w_t[0][:, h, :],
                        x_t[0][:, g, h, :],
                        start=(h == 0),
                        stop=False,
                    )
            for c in range(1, NC_):
                last = c == NC_ - 1
                for h in range(NH):
                    for g in range(NG):
                        nc.tensor.matmul(
                            ps[g][:, h * SB : (h + 1) * SB],
                            w_t[c][:, h, :],
                            x_t[c][:, g, h, :],
                            start=False,
                            stop=(last and h == NH - 1),
                        )
        else:
            for c in range(NC_):
                for g in range(NG):
                    for h in range(NH):
                        nc.tensor.matmul(
                            ps[g][:, h * NW : (h + 1) * NW],
                            x_t[c][:, g, h, :],
                            w_t[c][:, h, :],
                            start=(c == 0 and h == 0),
                            stop=(c == NC_ - 1 and h == NH - 1),
                        )

        for g in range(NG):
            ot = op.tile([SB, NI], F32, tag="ot")
            # Alternate evacuation between VectorE and ScalarE so two
            # banks drain at a time.
            if g % 2 == 1:
                nc.scalar.copy(ot[:], ps[g][:])
            else:
                nc.vector.tensor_copy(ot[:], ps[g][:])
            nc.sync.dma_start(out=out[g], in_=ot[:])

    nc.compile()
    _STATE["nc"] = nc
    return nc


def _shard(x, W):
    np_dt = mybir.dt.np(MM_DT)
    x = np.ascontiguousarray(np.asarray(x, dtype=np.float32)).astype(np_dt)
    W = np.ascontiguousarray(np.asarray(W, dtype=np.float32)).astype(np_dt)
    # xt[t][c, p, g, h, m] = x[QB*g + m//RW, 128t + RW*h + (m%RW), 128c + p]
    xr = x.reshape(NG, QB, NT, NH, RW, NC_, KC)  # [g, qi, t, h, r, c, p]
    xts = np.ascontiguousarray(np.transpose(xr, (2, 5, 6, 0, 3, 1, 4))).reshape(
        NT, NC_, KC, NG, NH, SB
    )
    # wt[t][c, p, h, n] = W[128c + p, 512t + NW*h + n]
    wr = W.reshape(NC_, KC, NT, NH, NW)  # [c, p, t, h, n]
    wts = np.ascontiguousarray(np.transpose(wr, (2, 0, 1, 3, 4)))
    return [{"xt": xts[t], "wt": wts[t]} for t in range(NT)]


def kernel(x, W, _trace=False, _trace_kwargs=None):
    nc = _build()
    in_maps = _shard(x, W)
    res = run_bass_kernel_spmd(
        nc,
        in_maps,
        list(range(NT)),
        trace=_trace,
        **(_trace_kwargs or {}),
    )
    _STATE["last_run"] = res
    band = np.empty((B, NT * SB, 4), dtype=np.float32)
    for t in range(NT):
        blk = np.ascontiguousarray(res.results[t]["out"])  # (NG, 128, 512)
        e = blk.strides[2]
        if WSTAT:
            # Transposed blocks: value (g, qi, h, r, j) sits at
            # blk[g, 4r + j, 128h + RW*qi + r].
            v = np.lib.stride_tricks.as_strided(
                blk,
                shape=(NG, QB, NH, RW, 4),
                strides=(
                    blk.strides[0],
                    RW * e,
                    SB * e,
                    4 * blk.strides[1] + e,
                    blk.strides[1],
                ),
            )
        else:
            # Row m = RW*qi + r holds batch QB*g + qi, s-row 128t + RW*h
            # + r; band value j at block col NW*h + 4r + j.
            v = np.lib.stride_tricks.as_strided(
                blk,
                shape=(NG, QB, NH, RW, 4),
                strides=(
                    blk.strides[0],
                    RW * blk.strides[1],
                    NW * e,
                    blk.strides[1] + 4 * e,
                    e,
                ),
            )
        # [g, qi, h, r, j] -> b = QB*g + qi, s_rel = RW*h + r
        band[:, t * SB : (t + 1) * SB, :] = v.reshape(B, SB, 4)
    s_idx = np.arange(NT * SB)
    y = np.zeros((B, NT * SB, NT * SB, 4), dtype=np.float32)
    y[:, s_idx, s_idx, :] = band
    return y.reshape(B, NT * SB, NT * NI)


# revision 13
# speedup vs baseline: 1.1773x; 1.1773x over previous
"""ButterflyLinear Trainium2 kernel.

Math: out[b, s, i] = (sum_o x[b, s, o] * W[o, i]) * mask[s, i], with
mask[s, i] = 1 iff 4s <= i < 4s+4 (stride-4 band). The band makes the
output block-diagonal: s-rows [128t, 128t+128) only touch output columns
[512t, 512t+512) -- an 8x compute reduction vs the full matmul.

Sharding (8 cores): core t owns s-block t for all 16 batches
(tensor-parallel split of W columns; no inter-core communication).

Packing: a 32-row s-sub-block spans a 128-wide band window, identical
for every batch. The moving operand packs FOUR batches (N = 128 = 4
batches x 32 s-rows) against the 128-wide W window as the STATIONARY
(out = window-col x batch-row, i.e. transposed blocks) -- one weight
load serves 4 batch-group matmuls and W streams once per batch QUAD.
Each accumulation chain lives in a QUARTER PSUM bank; all 16 chains
(4 batch-quads x 4 sub-blocks) fit in 4 banks at once: single wave.
PSUM has_written is per-element: only the first matmul of each bank
uses start=True (clears the whole bank); later chains in that bank
start with start=False and overwrite their untouched quarter.

Per-core device program (all fp16 inputs, fp32 PSUM accumulate):
  - 8 o-chunks streamed (W chunk 128KB + x chunk 512KB); the first x
    chunk is split per batch-quad so matmuls start ~2us earlier
  - 128 matmuls (K=128, M=128, N=128), 4 per weight load
  - 4 full-bank copies (alternating VectorE/ScalarE) -> DMA raw blocks
Host extracts the 4-wide diagonal band from the transposed raw blocks
into the zero-filled (16, 1024, 4096) result.
"""

import os
import sys
from contextlib import ExitStack

import numpy as np

if "/opt/trn_rl_repo" not in sys.path:
    sys.path.insert(0, "/opt/trn_rl_repo")

import concourse.bass as bass  # noqa: E402
import concourse.tile as tile  # noqa: E402
from concourse import bacc, mybir  # noqa: E402
from concourse.bass_utils import run_bass_kernel_spmd  # noqa: E402

B = 16  # batch
NT = 8  # s-blocks == cores
SB = 128  # s rows per block
NC_ = 8  # o chunks
KC = 128  # o rows per chunk
NI = 512  # output columns per block
QB = int(os.environ.get("BFK_QB", "4"))  # batches packed per matmul
RW = SB // QB  # s-rows per sub-block
NH = QB  # sub-blocks per s-block (RW*NH = 128)
NW = 4 * RW  # W window per sub-block
NG = B // QB  # batch groups
WSTAT = os.environ.get("BFK_WSTAT", "0") == "1"  # W as stationary
SPLIT0 = os.environ.get("BFK_SPLIT0", "0") == "1"  # split first x chunk
OUT16 = os.environ.get("BFK_OUT16", "1") == "1"  # fp16 output blocks
NOSW = os.environ.get("BFK_NOSWDGE", "0") == "1"  # drop SWDGE scratch (breaks walrus)

# fp16 (11-bit mantissa) halves DMA traffic vs f32 and streams the PE at
# 1 cycle/row; accumulation stays fp32 in PSUM (band rel err ~3e-4).
_DT_CHOICES = {
    "f16": mybir.dt.float16,
    "f32r": mybir.dt.float32r,
    "f32": mybir.dt.float32,
}
MM_DT = _DT_CHOICES[os.environ.get("BFK_DT", "f16")]
F32 = mybir.dt.float32

_STATE: dict = {}


def _build():
    if "nc" in _STATE:
        return _STATE["nc"]

    OUT_DT = mybir.dt.float16 if OUT16 else F32
    kw = {"dynamic_dma_scratch_size": 0} if NOSW else {}
    nc = bacc.Bacc(
        "TRN2", target_bir_lowering=False, debug=False, num_devices=NT, **kw
    )
    xt = nc.dram_tensor(
        "xt", [NC_, KC, NG, NH, SB], MM_DT, kind="ExternalInput"
    ).ap()
    wt = nc.dram_tensor("wt", [NC_, KC, NH, NW], MM_DT, kind="ExternalInput").ap()
    out = nc.dram_tensor("out", [NG, SB, NI], OUT_DT, kind="ExternalOutput").ap()

    with tile.TileContext(nc) as tc, ExitStack() as ctx:
        wp = ctx.enter_context(tc.tile_pool(name="w", bufs=1))
        xp = ctx.enter_context(tc.tile_pool(name="x", bufs=1))
        pp = ctx.enter_context(tc.tile_pool(name="ps", bufs=8, space="PSUM"))
        op = ctx.enter_context(tc.tile_pool(name="o", bufs=6))

        w_t = []
        x_t = []
        for c in range(NC_):
            w = wp.tile([KC, NH, NW], MM_DT, tag=f"w{c}")
            nc.sync.dma_start(out=w[:], in_=wt[c])
            w_t.append(w)
            xc = xp.tile([KC, NG, NH, SB], MM_DT, tag=f"x{c}")
            if c == 0 and SPLIT0:
                # Per-quad sub-DMAs so the first matmuls start as soon as
                # the first 128KB lands instead of waiting for 512KB.
                for g in range(NG):
                    nc.sync.dma_start(out=xc[:, g], in_=xt[c, :, g])
            else:
                nc.sync.dma_start(out=xc[:], in_=xt[c])
            x_t.append(xc)

        ps = [pp.tile([SB, NI], F32, tag="ps", name=f"ps_{g}") for g in range(NG)]

        if WSTAT:
            # out^T blocks: stationary = W window (o, NW), moving = x pack
            # (o, 128). For c==0, loop g-outer so each batch-quad's chains
            # start as soon as its sub-DMA lands; afterwards h-outer so one
            # weight load serves all 4 quads.
            for g in range(NG):
                for h in range(NH):
                    nc.tensor.matmul(
                        ps[g][:, h * SB : (h + 1) * SB],
                        w_t[0][:, h, :],
                        x_t[0][:, g, h, :],
                        start=(h == 0),
                        stop=False,
                    )
            for c in range(1, NC_):
                last = c == NC_ - 1
                for h in range(NH):
                    for g in range(NG):
                        nc.tensor.matmul(
                            ps[g][:, h * SB : (h + 1) * SB],
                            w_t[c][:, h, :],
                            x_t[c][:, g, h, :],
                            start=False,
                            stop=(last and h == NH - 1),
                        )
        else:
            for c in range(NC_):
                for g in range(NG):
                    for h in range(NH):
                        nc.tensor.matmul(
                            ps[g][:, h * NW : (h + 1) * NW],
                            x_t[c][:, g, h, :],
                            w_t[c][:, h, :],
                            start=(c == 0 and h == 0),
                            stop=(c == NC_ - 1 and h == NH - 1),
                        )

        for g in range(NG):
            ot = op.tile([SB, NI], OUT_DT, tag="ot")
            # Alternate evacuation between VectorE and ScalarE so two
            # banks drain at a time.
            if g % 2 == 1:
                nc.scalar.copy(ot[:], ps[g][:])
            else:
                nc.vector.tensor_copy(ot[:], ps[g][:])
            nc.sync.dma_start(out=out[g], in_=ot[:])

    nc.compile()
    _STATE["nc"] = nc
    return nc


def _shard(x, W):
    np_dt = mybir.dt.np(MM_DT)
    x = np.ascontiguousarray(np.asarray(x, dtype=np.float32)).astype(np_dt)
    W = np.ascontiguousarray(np.asarray(W, dtype=np.float32)).astype(np_dt)
    # xt[t][c, p, g, h, m] = x[QB*g + m//RW, 128t + RW*h + (m%RW), 128c + p]
    xr = x.reshape(NG, QB, NT, NH, RW, NC_, KC)  # [g, qi, t, h, r, c, p]
    xts = np.ascontiguousarray(np.transpose(xr, (2, 5, 6, 0, 3, 1, 4))).reshape(
        NT, NC_, KC, NG, NH, SB
    )
    # wt[t][c, p, h, n] = W[128c + p, 512t + NW*h + n]
    wr = W.reshape(NC_, KC, NT, NH, NW)  # [c, p, t, h, n]
    wts = np.ascontiguousarray(np.transpose(wr, (2, 0, 1, 3, 4)))
    return [{"xt": xts[t], "wt": wts[t]} for t in range(NT)]


def kernel(x, W, _trace=False, _trace_kwargs=None):
    nc = _build()
    in_maps = _shard(x, W)
    res = run_bass_kernel_spmd(
        nc,
        in_maps,
        list(range(NT)),
        trace=_trace,
        **(_trace_kwargs or {}),
    )
    _STATE["last_run"] = res
    band = np.empty((B, NT * SB, 4), dtype=np.float32)
    for t in range(NT):
        blk = np.ascontiguousarray(
            res.results[t]["out"].astype(np.float32)
        )  # (NG, 128, 512)
        e = blk.strides[2]
        if WSTAT:
            # Transposed blocks: value (g, qi, h, r, j) sits at
            # blk[g, 4r + j, 128h + RW*qi + r].
            v = np.lib.stride_tricks.as_strided(
                blk,
                shape=(NG, QB, NH, RW, 4),
                strides=(
                    blk.strides[0],
                    RW * e,
                    SB * e,
                    4 * blk.strides[1] + e,
                    blk.strides[1],
                ),
            )
        else:
            # Row m = RW*qi + r holds batch QB*g + qi, s-row 128t + RW*h
            # + r; band value j at block col NW*h + 4r + j.
            v = np.lib.stride_tricks.as_strided(
                blk,
                shape=(NG, QB, NH, RW, 4),
                strides=(
                    blk.strides[0],
                    RW * blk.strides[1],
                    NW * e,
                    blk.strides[1] + 4 * e,
                    e,
                ),
            )
        # [g, qi, h, r, j] -> b = QB*g + qi, s_rel = RW*h + r
        band[:, t * SB : (t + 1) * SB, :] = v.reshape(B, SB, 4)
    s_idx = np.arange(NT * SB)
    y = np.zeros((B, NT * SB, NT * SB, 4), dtype=np.float32)
    y[:, s_idx, s_idx, :] = band
    return y.reshape(B, NT * SB, NT * NI)
